# revision 20
# baseline (speedup 1.0000x reference)
"""PhiHarmonicAttention (B=1, S=2048, D=2048, H=16, Dh=128) on 8 Trainium2 cores.

Sharding: tensor-parallel over heads - 2 heads per core.
  - Wq/Wk/Wv column-sliced (256 cols per core), Wo row-sliced (256 rows).
  - Each core computes q^T/k^T (RoPE'd) + v for its 2 heads, causal
    softmax(QK^T)V in transposed layout, and a partial x-out product with its
    Wo slice. Host sums the 8 partials (TP row-parallel reduction).

All matmuls in fp16 (full PE rate at any moving size, half the SBUF/DMA
traffic of f32r). PSUM accumulation stays fp32. Causal masking is done by
accumulating a constant strictly-lower-triangular -400 matrix onto the
diagonal score block with one extra PE matmul; exp(scale*(s-400)) then
underflows to exactly 0 in fp16, so no vector-engine mask multiply and no
max-subtraction is needed (scores ~ N(0,1), exp is safe).

Emission is software-pipelined per 512-wide chunk slot:
  prefetch xt(ci+2) | A(ci+1) subpasses | B(h0,ci) | B(h1,ci) | C(ci-1)
with stage A split into 2-bank subpasses (q0k0 / q1k1 / v01 / v23) so PSUM
fits dedicated pools: A=2, scores=2, po=1, pd=1, out=2 banks.
"""
import numpy as np
from contextlib import ExitStack, nullcontext

import concourse.bass as bass
import concourse.tile as tile
from concourse import bacc, mybir
from concourse.bass_utils import run_bass_kernel_spmd

S = 2048
D = 2048
H = 16
DH = 128
NCORES = 8
HPC = H // NCORES          # heads per core = 2
CW = HPC * DH              # weight col-slice per core = 256
NO = D // 128              # contraction chunks = 16
NJ = S // 512              # 512-wide table chunks = 4 (rope tables)
W = 512                    # seq chunk width
NCH = S // W               # 4 chunks
NB = S // 128              # 128-wide seq blocks = 16
SCALE = float(1.0 / np.sqrt(np.float32(DH)))
MASK_BIG = 400.0

ROT_FACTOR = (1.0 + 5.0 ** 0.5) / 2.0 - 1.0
ROPE_BASE = 10000.0

F32 = mybir.dt.float32
F16 = mybir.dt.float16


def _build_nc(reps=1, stages="ABC", unroll=1):
    nc = bacc.Bacc("TRN2", target_bir_lowering=False, debug=False, num_devices=NCORES)

    xt_d = nc.dram_tensor("xt", [D, S], F16, kind="ExternalInput").ap()
    wq_d = nc.dram_tensor("wq", [D, CW], F16, kind="ExternalInput").ap()
    wk_d = nc.dram_tensor("wk", [D, CW], F16, kind="ExternalInput").ap()
    wv_d = nc.dram_tensor("wv", [D, CW], F16, kind="ExternalInput").ap()
    wo_d = nc.dram_tensor("wo", [CW, D], F16, kind="ExternalInput").ap()
    rcu_d = nc.dram_tensor("ropecu", [DH, 512], F32, kind="ExternalInput").ap()
    rsu_d = nc.dram_tensor("ropesu", [DH, 512], F32, kind="ExternalInput").ap()
    rc512_d = nc.dram_tensor("ropec512", [DH, NJ], F32, kind="ExternalInput").ap()
    rs512_d = nc.dram_tensor("ropes512", [DH, NJ], F32, kind="ExternalInput").ap()
    onc_d = nc.dram_tensor("onescol", [128, 1], F16, kind="ExternalInput").ap()
    tri_d = nc.dram_tensor("trimask", [128, 128], F16, kind="ExternalInput").ap()
    idn_d = nc.dram_tensor("ident", [128, 128], F16, kind="ExternalInput").ap()
    out_d = nc.dram_tensor("out", [S, D], F16, kind="ExternalOutput").ap()

    with ExitStack() as ctx:
        tc = ctx.enter_context(tile.TileContext(nc))
        consts = ctx.enter_context(tc.tile_pool(name="consts", bufs=1))
        persist = ctx.enter_context(tc.tile_pool(name="persist", bufs=1))
        xw = ctx.enter_context(tc.tile_pool(name="xw", bufs=12))
        ptp = ctx.enter_context(tc.tile_pool(name="ptp", bufs=3))
        work = ctx.enter_context(tc.tile_pool(name="work", bufs=2))
        outp = ctx.enter_context(tc.tile_pool(name="outp", bufs=2))
        psA = ctx.enter_context(tc.tile_pool(name="psA", bufs=2, space="PSUM"))
        psS = ctx.enter_context(tc.tile_pool(name="psS", bufs=2, space="PSUM"))
        psB = ctx.enter_context(tc.tile_pool(name="psB", bufs=1, space="PSUM"))
        psC = ctx.enter_context(tc.tile_pool(name="psC", bufs=2, space="PSUM"))

        # ---- constants ----
        wq_s = consts.tile([128, NO, CW], F16, tag="wq")
        wk_s = consts.tile([128, NO, CW], F16, tag="wk")
        wv_s = consts.tile([128, NO, CW], F16, tag="wv")
        wo_s = consts.tile([128, HPC, D], F16, tag="wo")
        rc = consts.tile([DH, S], F16, tag="rc")
        rs = consts.tile([DH, S], F16, tag="rs")
        onc = consts.tile([128, 1], F16, tag="onc")
        tri = consts.tile([128, 128], F16, tag="tri")
        idn = consts.tile([128, 128], F16, tag="idn")
        rcu = consts.tile([DH, 512], F32, tag="rcu")
        rsu = consts.tile([DH, 512], F32, tag="rsu")
        rc512 = consts.tile([DH, NJ], F32, tag="rc512")
        rs512 = consts.tile([DH, NJ], F32, tag="rs512")

        nc.scalar.dma_start(wq_s[:], wq_d.rearrange("(o p) n -> p o n", p=128))
        nc.scalar.dma_start(wk_s[:], wk_d.rearrange("(o p) n -> p o n", p=128))
        nc.scalar.dma_start(wv_s[:], wv_d.rearrange("(o p) n -> p o n", p=128))
        nc.scalar.dma_start(wo_s[:], wo_d.rearrange("(h p) n -> p h n", p=128))
        nc.scalar.dma_start(rcu[:], rcu_d)
        nc.scalar.dma_start(rsu[:], rsu_d)
        nc.scalar.dma_start(rc512[:], rc512_d)
        nc.scalar.dma_start(rs512[:], rs512_d)
        nc.scalar.dma_start(onc[:], onc_d)
        nc.scalar.dma_start(tri[:], tri_d)
        nc.scalar.dma_start(idn[:], idn_d)

        # rc/rs fp16 tables: angle(s0+u) via cos(a+b) expansion
        for j in range(NJ):
            sl = slice(512 * j, 512 * (j + 1))
            tm = work.tile([128, 512], F32, tag="tf32")
            nc.vector.tensor_scalar_mul(tm[:], rsu[:], rs512[:, j:j + 1])
            nc.vector.scalar_tensor_tensor(
                rc[:, sl], rcu[:], rc512[:, j:j + 1], tm[:],
                mybir.AluOpType.mult, mybir.AluOpType.subtract,
            )
            tm2 = work.tile([128, 512], F32, tag="tf32b")
            nc.vector.tensor_scalar_mul(tm2[:], rcu[:], rs512[:, j:j + 1])
            nc.vector.scalar_tensor_tensor(
                rs[:, sl], rsu[:], rc512[:, j:j + 1], tm2[:],
                mybir.AluOpType.mult, mybir.AluOpType.add,
            )

        rep_ctx = (
            tc.For_i(
                0, reps, 1,
                hint_engines=tuple(
                    getattr(mybir.EngineType, e)
                    for e in ("PE", "DVE", "Activation", "SP", "Pool")
                ),
            )
            if reps > 1 else nullcontext()
        )
        ctx.enter_context(rep_ctx)

        # ---- persistent per-head tensors ----
        qT = [persist.tile([DH, S], F16, tag=f"qT{h}", name=f"qT{h}")
              for h in range(HPC)]
        kT = [persist.tile([DH, S], F16, tag=f"kT{h}", name=f"kT{h}")
              for h in range(HPC)]
        v_sb = persist.tile([128, NB, CW], F16, tag="v")
        aT = [persist.tile([DH, S], F16, tag=f"aT{h}", name=f"aT{h}")
              for h in range(HPC)]

        def prefetch_xt(ci):
            """4 batched DMAs of [128, 4o, 512] fp16 for chunk ci."""
            s0 = W * ci
            tiles = []
            for g in range(4):
                t = xw.tile([128, 4, W], F16, tag="xt", name=f"xt{ci}_{g}")
                nc.sync.dma_start(
                    t[:],
                    xt_d[512 * g:512 * (g + 1), s0:s0 + W].rearrange(
                        "(o p) s -> p o s", p=128
                    ),
                )
                tiles.append(t)
            return tiles

        def rope_apply(psum, dst_slice, s0, w):
            cs = rc[:, s0:s0 + w]
            sn = rs[:, s0:s0 + w]
            raw = work.tile([128, 512], F16, tag="raw")
            nc.scalar.copy(raw[:, :w], psum[:])      # frees the PSUM bank fast
            t1 = work.tile([128, 512], F16, tag="t1")
            nc.vector.tensor_mul(t1[:, :w], raw[:, :w], cs)
            tsw = work.tile([128, 512], F16, tag="tsw")
            nc.vector.tensor_copy(tsw[0:64, :w], raw[64:128, :w])
            nc.vector.tensor_copy(tsw[64:128, :w], raw[0:64, :w])
            nc.vector.tensor_mul(tsw[:, :w], tsw[:, :w], sn)
            nc.vector.tensor_add(dst_slice, t1[:, :w], tsw[:, :w])

        def a_units(ci, xts):
            """Stage A as 64 fine steps (one per contraction o-chunk) so the
            weave can slot projection matmuls into attention's exp-wait gaps.
            Subpasses q0k0, q1k1, v01, v23 rotate on a 2-bank PSUM ring."""
            s0 = W * ci

            def xt(o):
                return xts[o // 4][:, o % 4, :]

            def qk_step(h, o, box):
                if o == 0:
                    box["pq"] = psA.tile([128, W], F32, tag="pa",
                                         name=f"pq{ci}_{h}")
                    box["pk"] = psA.tile([128, W], F32, tag="pa",
                                         name=f"pk{ci}_{h}")
                st = dict(start=(o == 0), stop=(o == NO - 1))
                nc.tensor.matmul(
                    box["pq"][:], wq_s[:, o, 128 * h:128 * (h + 1)], xt(o), **st
                )
                nc.tensor.matmul(
                    box["pk"][:], wk_s[:, o, 128 * h:128 * (h + 1)], xt(o), **st
                )
                if o == NO - 1:
                    rope_apply(box["pq"], qT[h][:, s0:s0 + W], s0, W)
                    rope_apply(box["pk"], kT[h][:, s0:s0 + W], s0, W)

            def v_step(m0, o, box):
                if o == 0:
                    box["pv"] = [
                        psA.tile([128, W], F32, tag="pa",
                                 name=f"pv{ci}_{m0}_{i}")
                        for i in range(2)
                    ]
                st = dict(start=(o == 0), stop=(o == NO - 1))
                for i in range(2):
                    m4 = m0 + i
                    nc.tensor.matmul(
                        box["pv"][i][:, :CW],
                        xt(o)[:, 128 * m4:128 * (m4 + 1)], wv_s[:, o, :],
                        **st
                    )
                if o == NO - 1:
                    for i in range(2):
                        nc.vector.tensor_copy(
                            v_sb[:, s0 // 128 + m0 + i, :],
                            box["pv"][i][:, :CW],
                        )

            for h in range(HPC):
                box = {}
                for o in range(NO):
                    yield lambda h=h, o=o, box=box: qk_step(h, o, box)
            for m0 in (0, 2):
                box = {}
                for o in range(NO):
                    yield lambda m0=m0, o=o, box=box: v_step(m0, o, box)

        def b_units(h, ci):
            """Stage B as nb+1 fine units: unit b emits pss(b)+exp(b), then
            pd/po of block b-1 (one-block score lookahead on a 2-bank ring)."""
            s0 = W * ci
            nb = (s0 + W) // 128
            sb0 = s0 // 128
            state = {}

            def emit_pss(b):
                r = b - sb0
                c0 = 128 * r if r > 0 else 0
                pss = psS.tile([128, W], F32, tag="pss", name=f"pss{h}_{ci}_{b}")
                nc.tensor.matmul(
                    pss[:, c0:W],
                    kT[h][:, 128 * b:128 * (b + 1)],
                    qT[h][:, s0 + c0:s0 + W],
                    start=True,
                    stop=(r < 0),
                )
                if r >= 0:
                    # strictly-lower-tri -400 onto the diagonal 128x128 block;
                    # exp then underflows masked entries to exactly 0 in fp16
                    nc.tensor.matmul(
                        pss[:, 128 * r:128 * (r + 1)], idn[:], tri[:],
                        start=False, stop=True,
                    )
                pt = ptp.tile([128, W], F16, tag="pt")
                nc.scalar.activation(
                    pt[:, c0:W], pss[:, c0:W],
                    mybir.ActivationFunctionType.Exp, scale=SCALE,
                )
                return pt, c0

            def emit_dpo(b, pt, c0):
                stv = dict(start=(b == 0), stop=(b == nb - 1))
                nc.tensor.matmul(pd[:, c0:], onc[:], pt[:, c0:W], **stv)
                nc.tensor.matmul(
                    po[:, c0:], v_sb[:, b, 128 * h:128 * (h + 1)], pt[:, c0:W],
                    **stv
                )

            po = psB.tile([128, W], F32, tag="po", name=f"po{h}_{ci}")
            pd = psB.tile([1, W], F32, tag="pd", name=f"pd{h}_{ci}")

            def unit(b):
                if b < nb:
                    cur = (b,) + emit_pss(b)
                    if b > 0:
                        emit_dpo(*state["prev"])
                    state["prev"] = cur
                    return
                emit_dpo(*state["prev"])
                rec = work.tile([1, W], F32, tag="rec", bufs=1)
                with nc.allow_low_precision("softmax denom recip"):
                    nc.vector.reciprocal(rec[:], pd[:])
                bc = work.tile([128, W], F32, tag="bc")
                nc.gpsimd.partition_broadcast(bc[:], rec[:])
                nc.vector.tensor_mul(aT[h][:, s0:s0 + W], po[:], bc[:])

            for b in range(nb + 1):
                yield lambda b=b: unit(b)

        def c_units(ci):
            """Stage C as 16 fine units (one per 512-wide output tile)."""
            s0 = W * ci
            state = {}

            def unit(m4, e):
                m = s0 // 128 + m4
                if e == 0:
                    state["ot"] = outp.tile([128, NJ, 512], F16, tag="ot",
                                            name=f"ot{ci}_{m4}")
                ot = state["ot"]
                pf = psC.tile([128, 512], F32, tag="pf",
                              name=f"pf{ci}_{m4}_{e}")
                for h2 in range(HPC):
                    nc.tensor.matmul(
                        pf[:],
                        aT[h2][:, 128 * m:128 * (m + 1)],
                        wo_s[:, h2, 512 * e:512 * (e + 1)],
                        start=(h2 == 0),
                        stop=(h2 == HPC - 1),
                    )
                if e % 2 == 0:
                    nc.vector.tensor_copy(ot[:, e, :], pf[:])
                else:
                    nc.scalar.copy(ot[:, e, :], pf[:])
                if e == NJ - 1:
                    nc.sync.dma_start(
                        out_d[128 * m:128 * (m + 1), :],
                        ot[:].rearrange("p e n -> p (e n)"),
                    )

            for m4 in range(4):
                for e in range(NJ):
                    yield lambda m4=m4, e=e: unit(m4, e)

        def weave(main, filler):
            """Merge two unit lists, spreading filler evenly through main."""
            out = []
            i = j = 0
            nm, nf = len(main), len(filler)
            while i < nm or j < nf:
                if j >= nf or (i < nm and i * nf <= j * nm):
                    out.append(main[i])
                    i += 1
                else:
                    out.append(filler[j])
                    j += 1
            return out

        def run_units(units):
            for u in units:
                u()

        # ---- software-pipelined chunk slots, instruction-level weave ----
        # Inside the rep loop, stage C of the LAST chunk is rotated into the
        # next body's prologue (woven with A(0)) so its PSUM-ring waits never
        # sit at the head of the in-order PE queue. First iteration reads
        # stale aT for it (bench-only path; out rows are rewritten); the
        # reps=1 correctness path keeps C(3) at the body end, unrotated.
        rotate = reps > 1 or unroll > 1
        for _u in range(unroll):
            xts = {}
            xts[0] = prefetch_xt(0)
            xts[1] = prefetch_xt(1)
            prol = list(a_units(0, xts[0])) if "A" in stages else []
            if rotate and "C" in stages:
                prol = weave(prol, list(c_units(NCH - 1)))
            run_units(prol)
            for ci in range(NCH):
                if ci + 2 < NCH:
                    xts[ci + 2] = prefetch_xt(ci + 2)
                au = (
                    list(a_units(ci + 1, xts[ci + 1]))
                    if ("A" in stages and ci + 1 < NCH) else []
                )
                cu = list(c_units(ci - 1)) if ("C" in stages and ci >= 1) else []
                main = (
                    list(b_units(0, ci)) + list(b_units(1, ci))
                    if "B" in stages else []
                )
                run_units(weave(main, au + cu))
            if not rotate and "C" in stages:
                run_units(list(c_units(NCH - 1)))

    nc.compile()
    return nc


def _host_inputs(x, Wq, Wk, Wv, Wo):
    x = np.asarray(x, dtype=np.float32).reshape(S, D)
    xt = np.ascontiguousarray(x.T).astype(np.float16)

    half = DH // 2
    inv_freq = (
        ROT_FACTOR
        / (ROPE_BASE ** (np.arange(0, half, dtype=np.float32) * 2.0 / DH))
    ).astype(np.float32)
    sgn = np.where(np.arange(DH) < half, -1.0, 1.0).astype(np.float32)[:, None]
    fd = np.concatenate([inv_freq, inv_freq]).astype(np.float32)[:, None]  # [128,1]
    u = np.arange(512, dtype=np.float32)[None, :]
    jj = (512.0 * np.arange(NJ, dtype=np.float32))[None, :]
    ropecu = np.cos(fd * u).astype(np.float32)            # [128, 512]
    ropesu = (sgn * np.sin(fd * u)).astype(np.float32)
    ropec512 = np.cos(fd * jj).astype(np.float32)         # [128, NJ]
    ropes512 = (sgn * np.sin(fd * jj)).astype(np.float32)

    onescol = np.ones((128, 1), dtype=np.float16)
    rr = np.arange(128)
    trimask = np.where(
        rr[:, None] > rr[None, :], -MASK_BIG, 0.0
    ).astype(np.float16)
    ident = np.eye(128, dtype=np.float16)

    Wq = np.asarray(Wq, dtype=np.float32)
    Wk = np.asarray(Wk, dtype=np.float32)
    Wv = np.asarray(Wv, dtype=np.float32)
    Wo = np.asarray(Wo, dtype=np.float32)

    in_maps = []
    for c in range(NCORES):
        sl = slice(CW * c, CW * (c + 1))
        in_maps.append(
            {
                "xt": xt,
                "wq": np.ascontiguousarray(Wq[:, sl]).astype(np.float16),
                "wk": np.ascontiguousarray(Wk[:, sl]).astype(np.float16),
                "wv": np.ascontiguousarray(Wv[:, sl]).astype(np.float16),
                "wo": np.ascontiguousarray(Wo[sl, :]).astype(np.float16),
                "ropecu": ropecu,
                "ropesu": ropesu,
                "ropec512": ropec512,
                "ropes512": ropes512,
                "onescol": onescol,
                "trimask": trimask,
                "ident": ident,
            }
        )
    return in_maps


_NC_CACHE = None


def kernel(x, Wq, Wk, Wv, Wo):
    global _NC_CACHE
    if _NC_CACHE is None:
        _NC_CACHE = _build_nc()
    in_maps = _host_inputs(x, Wq, Wk, Wv, Wo)
    res = run_bass_kernel_spmd(_NC_CACHE, in_maps, core_ids=list(range(NCORES)))
    out = np.zeros((S, D), dtype=np.float32)
    for r in res.results:
        out += r["out"].astype(np.float32)
    return out.reshape(1, S, D)


# revision 21
# speedup vs baseline: 1.1437x; 1.1437x over previous
"""PhiHarmonicAttention (B=1, S=2048, D=2048, H=16, Dh=128) on 8 Trainium2 cores.

Sharding: tensor-parallel over heads - 2 heads per core.
  - Wq/Wk/Wv column-sliced (256 cols per core), Wo row-sliced (256 rows).
  - Each core computes q^T/k^T (RoPE'd) + v for its 2 heads, causal
    softmax(QK^T)V in transposed layout, and a partial x-out product with its
    Wo slice. Host sums the 8 partials (TP row-parallel reduction).

All matmuls in fp16 (full PE rate at any moving size, half the SBUF/DMA
traffic of f32r). PSUM accumulation stays fp32. Causal masking is done by
accumulating a constant strictly-lower-triangular -400 matrix onto the
diagonal score block with one extra PE matmul; exp(scale*(s-400)) then
underflows to exactly 0 in fp16, so no vector-engine mask multiply and no
max-subtraction is needed (scores ~ N(0,1), exp is safe).

Emission is software-pipelined per 512-wide chunk slot:
  prefetch xt(ci+2) | A(ci+1) subpasses | B(h0,ci) | B(h1,ci) | C(ci-1)
with stage A split into 2-bank subpasses (q0k0 / q1k1 / v01 / v23) so PSUM
fits dedicated pools: A=2, scores=2, po=1, pd=1, out=2 banks.
"""
import numpy as np
from contextlib import ExitStack, nullcontext

import concourse.bass as bass
import concourse.tile as tile
from concourse import bacc, mybir
from concourse.bass_utils import run_bass_kernel_spmd

S = 2048
D = 2048
H = 16
DH = 128
NCORES = 8
HPC = H // NCORES          # heads per core = 2
CW = HPC * DH              # weight col-slice per core = 256
NO = D // 128              # contraction chunks = 16
NJ = S // 512              # 512-wide table chunks = 4 (rope tables)
W = 512                    # seq chunk width
NCH = S // W               # 4 chunks
NB = S // 128              # 128-wide seq blocks = 16
SCALE = float(1.0 / np.sqrt(np.float32(DH)))
MASK_BIG = 400.0

ROT_FACTOR = (1.0 + 5.0 ** 0.5) / 2.0 - 1.0
ROPE_BASE = 10000.0

F32 = mybir.dt.float32
F16 = mybir.dt.float16


def _build_nc(reps=1, stages="ABC", unroll=1):
    nc = bacc.Bacc("TRN2", target_bir_lowering=False, debug=False, num_devices=NCORES)

    xt_d = nc.dram_tensor("xt", [D, S], F16, kind="ExternalInput").ap()
    wq_d = nc.dram_tensor("wq", [D, CW], F16, kind="ExternalInput").ap()
    wk_d = nc.dram_tensor("wk", [D, CW], F16, kind="ExternalInput").ap()
    wv_d = nc.dram_tensor("wv", [D, CW], F16, kind="ExternalInput").ap()
    wo_d = nc.dram_tensor("wo", [CW, D], F16, kind="ExternalInput").ap()
    rcu_d = nc.dram_tensor("ropecu", [DH, 512], F32, kind="ExternalInput").ap()
    rsu_d = nc.dram_tensor("ropesu", [DH, 512], F32, kind="ExternalInput").ap()
    rc512_d = nc.dram_tensor("ropec512", [DH, NJ], F32, kind="ExternalInput").ap()
    rs512_d = nc.dram_tensor("ropes512", [DH, NJ], F32, kind="ExternalInput").ap()
    onc_d = nc.dram_tensor("onescol", [128, 1], F16, kind="ExternalInput").ap()
    tri_d = nc.dram_tensor("trimask", [128, 128], F16, kind="ExternalInput").ap()
    idn_d = nc.dram_tensor("ident", [128, 128], F16, kind="ExternalInput").ap()
    out_d = nc.dram_tensor("out", [S, D], F16, kind="ExternalOutput").ap()

    with ExitStack() as ctx:
        tc = ctx.enter_context(tile.TileContext(nc))
        consts = ctx.enter_context(tc.tile_pool(name="consts", bufs=1))
        persist = ctx.enter_context(tc.tile_pool(name="persist", bufs=1))
        xw = ctx.enter_context(tc.tile_pool(name="xw", bufs=12))
        ptp = ctx.enter_context(tc.tile_pool(name="ptp", bufs=4))
        work = ctx.enter_context(tc.tile_pool(name="work", bufs=3))
        outp = ctx.enter_context(tc.tile_pool(name="outp", bufs=3))
        psA = ctx.enter_context(tc.tile_pool(name="psA", bufs=2, space="PSUM"))
        psS = ctx.enter_context(tc.tile_pool(name="psS", bufs=2, space="PSUM"))
        psB = ctx.enter_context(tc.tile_pool(name="psB", bufs=1, space="PSUM"))
        psC = ctx.enter_context(tc.tile_pool(name="psC", bufs=2, space="PSUM"))

        # ---- constants ----
        wq_s = consts.tile([128, NO, CW], F16, tag="wq")
        wk_s = consts.tile([128, NO, CW], F16, tag="wk")
        wv_s = consts.tile([128, NO, CW], F16, tag="wv")
        wo_s = consts.tile([128, HPC, D], F16, tag="wo")
        rc = consts.tile([DH, S], F16, tag="rc")
        rs = consts.tile([DH, S], F16, tag="rs")
        onc = consts.tile([128, 1], F16, tag="onc")
        tri = consts.tile([128, 128], F16, tag="tri")
        idn = consts.tile([128, 128], F16, tag="idn")
        rcu = consts.tile([DH, 512], F32, tag="rcu")
        rsu = consts.tile([DH, 512], F32, tag="rsu")
        rc512 = consts.tile([DH, NJ], F32, tag="rc512")
        rs512 = consts.tile([DH, NJ], F32, tag="rs512")

        nc.scalar.dma_start(wq_s[:], wq_d.rearrange("(o p) n -> p o n", p=128))
        nc.scalar.dma_start(wk_s[:], wk_d.rearrange("(o p) n -> p o n", p=128))
        nc.scalar.dma_start(wv_s[:], wv_d.rearrange("(o p) n -> p o n", p=128))
        nc.scalar.dma_start(wo_s[:], wo_d.rearrange("(h p) n -> p h n", p=128))
        nc.scalar.dma_start(rcu[:], rcu_d)
        nc.scalar.dma_start(rsu[:], rsu_d)
        nc.scalar.dma_start(rc512[:], rc512_d)
        nc.scalar.dma_start(rs512[:], rs512_d)
        nc.scalar.dma_start(onc[:], onc_d)
        nc.scalar.dma_start(tri[:], tri_d)
        nc.scalar.dma_start(idn[:], idn_d)

        # rc/rs fp16 tables: angle(s0+u) via cos(a+b) expansion
        for j in range(NJ):
            sl = slice(512 * j, 512 * (j + 1))
            tm = work.tile([128, 512], F32, tag="tf32")
            nc.vector.tensor_scalar_mul(tm[:], rsu[:], rs512[:, j:j + 1])
            nc.vector.scalar_tensor_tensor(
                rc[:, sl], rcu[:], rc512[:, j:j + 1], tm[:],
                mybir.AluOpType.mult, mybir.AluOpType.subtract,
            )
            tm2 = work.tile([128, 512], F32, tag="tf32b")
            nc.vector.tensor_scalar_mul(tm2[:], rcu[:], rs512[:, j:j + 1])
            nc.vector.scalar_tensor_tensor(
                rs[:, sl], rsu[:], rc512[:, j:j + 1], tm2[:],
                mybir.AluOpType.mult, mybir.AluOpType.add,
            )

        rep_ctx = (
            tc.For_i(
                0, reps, 1,
                hint_engines=tuple(
                    getattr(mybir.EngineType, e)
                    for e in ("PE", "DVE", "Activation", "SP", "Pool")
                ),
            )
            if reps > 1 else nullcontext()
        )
        ctx.enter_context(rep_ctx)

        # ---- persistent per-head tensors ----
        qT = [persist.tile([DH, S], F16, tag=f"qT{h}", name=f"qT{h}")
              for h in range(HPC)]
        kT = [persist.tile([DH, S], F16, tag=f"kT{h}", name=f"kT{h}")
              for h in range(HPC)]
        v_sb = persist.tile([128, NB, CW], F16, tag="v")
        aT = [persist.tile([DH, S], F16, tag=f"aT{h}", name=f"aT{h}")
              for h in range(HPC)]

        def prefetch_xt(ci):
            """4 batched DMAs of [128, 4o, 512] fp16 for chunk ci."""
            s0 = W * ci
            tiles = []
            for g in range(4):
                t = xw.tile([128, 4, W], F16, tag="xt", name=f"xt{ci}_{g}")
                nc.sync.dma_start(
                    t[:],
                    xt_d[512 * g:512 * (g + 1), s0:s0 + W].rearrange(
                        "(o p) s -> p o s", p=128
                    ),
                )
                tiles.append(t)
            return tiles

        def rope_apply(psum, dst_slice, s0, w):
            cs = rc[:, s0:s0 + w]
            sn = rs[:, s0:s0 + w]
            raw = work.tile([128, 512], F16, tag="raw")
            nc.scalar.copy(raw[:, :w], psum[:])      # frees the PSUM bank fast
            t1 = work.tile([128, 512], F16, tag="t1")
            nc.vector.tensor_mul(t1[:, :w], raw[:, :w], cs)
            tsw = work.tile([128, 512], F16, tag="tsw")
            nc.vector.tensor_copy(tsw[0:64, :w], raw[64:128, :w])
            nc.vector.tensor_copy(tsw[64:128, :w], raw[0:64, :w])
            nc.vector.tensor_mul(tsw[:, :w], tsw[:, :w], sn)
            nc.vector.tensor_add(dst_slice, t1[:, :w], tsw[:, :w])

        def a_units(ci, xts):
            """Stage A as 64 fine steps (one per contraction o-chunk) so the
            weave can slot projection matmuls into attention's exp-wait gaps.
            Subpasses q0k0, q1k1, v01, v23 rotate on a 2-bank PSUM ring."""
            s0 = W * ci

            def xt(o):
                return xts[o // 4][:, o % 4, :]

            def qk_step(h, o, box):
                if o == 0:
                    box["pq"] = psA.tile([128, W], F32, tag="pa",
                                         name=f"pq{ci}_{h}")
                    box["pk"] = psA.tile([128, W], F32, tag="pa",
                                         name=f"pk{ci}_{h}")
                st = dict(start=(o == 0), stop=(o == NO - 1))
                nc.tensor.matmul(
                    box["pq"][:], wq_s[:, o, 128 * h:128 * (h + 1)], xt(o), **st
                )
                nc.tensor.matmul(
                    box["pk"][:], wk_s[:, o, 128 * h:128 * (h + 1)], xt(o), **st
                )
                if o == NO - 1:
                    rope_apply(box["pq"], qT[h][:, s0:s0 + W], s0, W)
                    rope_apply(box["pk"], kT[h][:, s0:s0 + W], s0, W)

            def v_step(m0, o, box):
                if o == 0:
                    box["pv"] = [
                        psA.tile([128, W], F32, tag="pa",
                                 name=f"pv{ci}_{m0}_{i}")
                        for i in range(2)
                    ]
                st = dict(start=(o == 0), stop=(o == NO - 1))
                for i in range(2):
                    m4 = m0 + i
                    nc.tensor.matmul(
                        box["pv"][i][:, :CW],
                        xt(o)[:, 128 * m4:128 * (m4 + 1)], wv_s[:, o, :],
                        **st
                    )
                if o == NO - 1:
                    for i in range(2):
                        nc.vector.tensor_copy(
                            v_sb[:, s0 // 128 + m0 + i, :],
                            box["pv"][i][:, :CW],
                        )

            for h in range(HPC):
                box = {}
                for o in range(NO):
                    yield lambda h=h, o=o, box=box: qk_step(h, o, box)
            for m0 in (0, 2):
                box = {}
                for o in range(NO):
                    yield lambda m0=m0, o=o, box=box: v_step(m0, o, box)

        def b_units(h, ci):
            """Stage B as nb+1 fine units: unit b emits pss(b)+exp(b), then
            pd/po of block b-1 (one-block score lookahead on a 2-bank ring)."""
            s0 = W * ci
            nb = (s0 + W) // 128
            sb0 = s0 // 128
            state = {}

            def emit_pss(b):
                r = b - sb0
                c0 = 128 * r if r > 0 else 0
                pss = psS.tile([128, W], F32, tag="pss", name=f"pss{h}_{ci}_{b}")
                nc.tensor.matmul(
                    pss[:, c0:W],
                    kT[h][:, 128 * b:128 * (b + 1)],
                    qT[h][:, s0 + c0:s0 + W],
                    start=True,
                    stop=(r < 0),
                )
                if r >= 0:
                    # strictly-lower-tri -400 onto the diagonal 128x128 block;
                    # exp then underflows masked entries to exactly 0 in fp16
                    nc.tensor.matmul(
                        pss[:, 128 * r:128 * (r + 1)], idn[:], tri[:],
                        start=False, stop=True,
                    )
                pt = ptp.tile([128, W], F16, tag="pt")
                nc.scalar.activation(
                    pt[:, c0:W], pss[:, c0:W],
                    mybir.ActivationFunctionType.Exp, scale=SCALE,
                )
                return pt, c0

            def emit_dpo(b, pt, c0):
                stv = dict(start=(b == 0), stop=(b == nb - 1))
                nc.tensor.matmul(pd[:, c0:], onc[:], pt[:, c0:W], **stv)
                nc.tensor.matmul(
                    po[:, c0:], v_sb[:, b, 128 * h:128 * (h + 1)], pt[:, c0:W],
                    **stv
                )

            po = psB.tile([128, W], F32, tag="po", name=f"po{h}_{ci}")
            pd = psB.tile([1, W], F32, tag="pd", name=f"pd{h}_{ci}")

            def unit(b):
                if b < nb:
                    cur = (b,) + emit_pss(b)
                    if b > 0:
                        emit_dpo(*state["prev"])
                    state["prev"] = cur
                    return
                emit_dpo(*state["prev"])
                rec = work.tile([1, W], F32, tag="rec", bufs=1)
                with nc.allow_low_precision("softmax denom recip"):
                    nc.vector.reciprocal(rec[:], pd[:])
                bc = work.tile([128, W], F32, tag="bc")
                nc.gpsimd.partition_broadcast(bc[:], rec[:])
                nc.vector.tensor_mul(aT[h][:, s0:s0 + W], po[:], bc[:])

            for b in range(nb + 1):
                yield lambda b=b: unit(b)

        def c_units(ci):
            """Stage C as 16 fine units (one per 512-wide output tile)."""
            s0 = W * ci
            state = {}

            def unit(m4, e):
                m = s0 // 128 + m4
                if e == 0:
                    state["ot"] = outp.tile([128, NJ, 512], F16, tag="ot",
                                            name=f"ot{ci}_{m4}")
                ot = state["ot"]
                pf = psC.tile([128, 512], F32, tag="pf",
                              name=f"pf{ci}_{m4}_{e}")
                for h2 in range(HPC):
                    nc.tensor.matmul(
                        pf[:],
                        aT[h2][:, 128 * m:128 * (m + 1)],
                        wo_s[:, h2, 512 * e:512 * (e + 1)],
                        start=(h2 == 0),
                        stop=(h2 == HPC - 1),
                    )
                if e % 2 == 0:
                    nc.vector.tensor_copy(ot[:, e, :], pf[:])
                else:
                    nc.scalar.copy(ot[:, e, :], pf[:])
                if e == NJ - 1:
                    nc.sync.dma_start(
                        out_d[128 * m:128 * (m + 1), :],
                        ot[:].rearrange("p e n -> p (e n)"),
                    )

            for m4 in range(4):
                for e in range(NJ):
                    yield lambda m4=m4, e=e: unit(m4, e)

        def weave(main, filler):
            """Merge two unit lists, spreading filler evenly through main."""
            out = []
            i = j = 0
            nm, nf = len(main), len(filler)
            while i < nm or j < nf:
                if j >= nf or (i < nm and i * nf <= j * nm):
                    out.append(main[i])
                    i += 1
                else:
                    out.append(filler[j])
                    j += 1
            return out

        def run_units(units):
            for u in units:
                u()

        # ---- software-pipelined chunk slots, instruction-level weave ----
        # Inside the rep loop, stage C of the LAST chunk is rotated into the
        # next body's prologue (woven with A(0)) so its PSUM-ring waits never
        # sit at the head of the in-order PE queue. First iteration reads
        # stale aT for it (bench-only path; out rows are rewritten); the
        # reps=1 correctness path keeps C(3) at the body end, unrotated.
        rotate = reps > 1 or unroll > 1
        for _u in range(unroll):
            xts = {}
            xts[0] = prefetch_xt(0)
            xts[1] = prefetch_xt(1)
            prol = list(a_units(0, xts[0])) if "A" in stages else []
            if rotate and "C" in stages:
                prol = weave(prol, list(c_units(NCH - 1)))
            run_units(prol)
            for ci in range(NCH):
                if ci + 2 < NCH:
                    xts[ci + 2] = prefetch_xt(ci + 2)
                au = (
                    list(a_units(ci + 1, xts[ci + 1]))
                    if ("A" in stages and ci + 1 < NCH) else []
                )
                cu = list(c_units(ci - 1)) if ("C" in stages and ci >= 1) else []
                main = (
                    list(b_units(0, ci)) + list(b_units(1, ci))
                    if "B" in stages else []
                )
                run_units(weave(main, au + cu))
            if not rotate and "C" in stages:
                run_units(list(c_units(NCH - 1)))

    nc.compile()
    return nc


def _host_inputs(x, Wq, Wk, Wv, Wo):
    x = np.asarray(x, dtype=np.float32).reshape(S, D)
    xt = np.ascontiguousarray(x.T).astype(np.float16)

    half = DH // 2
    inv_freq = (
        ROT_FACTOR
        / (ROPE_BASE ** (np.arange(0, half, dtype=np.float32) * 2.0 / DH))
    ).astype(np.float32)
    sgn = np.where(np.arange(DH) < half, -1.0, 1.0).astype(np.float32)[:, None]
    fd = np.concatenate([inv_freq, inv_freq]).astype(np.float32)[:, None]  # [128,1]
    u = np.arange(512, dtype=np.float32)[None, :]
    jj = (512.0 * np.arange(NJ, dtype=np.float32))[None, :]
    ropecu = np.cos(fd * u).astype(np.float32)            # [128, 512]
    ropesu = (sgn * np.sin(fd * u)).astype(np.float32)
    ropec512 = np.cos(fd * jj).astype(np.float32)         # [128, NJ]
    ropes512 = (sgn * np.sin(fd * jj)).astype(np.float32)

    onescol = np.ones((128, 1), dtype=np.float16)
    rr = np.arange(128)
    trimask = np.where(
        rr[:, None] > rr[None, :], -MASK_BIG, 0.0
    ).astype(np.float16)
    ident = np.eye(128, dtype=np.float16)

    Wq = np.asarray(Wq, dtype=np.float32)
    Wk = np.asarray(Wk, dtype=np.float32)
    Wv = np.asarray(Wv, dtype=np.float32)
    Wo = np.asarray(Wo, dtype=np.float32)

    in_maps = []
    for c in range(NCORES):
        sl = slice(CW * c, CW * (c + 1))
        in_maps.append(
            {
                "xt": xt,
                "wq": np.ascontiguousarray(Wq[:, sl]).astype(np.float16),
                "wk": np.ascontiguousarray(Wk[:, sl]).astype(np.float16),
                "wv": np.ascontiguousarray(Wv[:, sl]).astype(np.float16),
                "wo": np.ascontiguousarray(Wo[sl, :]).astype(np.float16),
                "ropecu": ropecu,
                "ropesu": ropesu,
                "ropec512": ropec512,
                "ropes512": ropes512,
                "onescol": onescol,
                "trimask": trimask,
                "ident": ident,
            }
        )
    return in_maps


_NC_CACHE = None


def kernel(x, Wq, Wk, Wv, Wo):
    global _NC_CACHE
    if _NC_CACHE is None:
        _NC_CACHE = _build_nc()
    in_maps = _host_inputs(x, Wq, Wk, Wv, Wo)
    res = run_bass_kernel_spmd(_NC_CACHE, in_maps, core_ids=list(range(NCORES)))
    out = np.zeros((S, D), dtype=np.float32)
    for r in res.results:
        out += r["out"].astype(np.float32)
    return out.reshape(1, S, D)
